# revision 28
# baseline (speedup 1.0000x reference)
"""Causal self-attention (GPT-style, B=2, T=4096, C=768, 12 heads) on 8 TRN2
NeuronCores.

Sharding: core c handles batch b = c//4 and heads [3g, 3g+1, 3g+2] with
g = c%4 (data parallel on B x tensor parallel on heads).  Each core computes
its heads' attention output projected through its slice of w_proj; the host
sums the 4 partial [T, C] outputs per batch and adds b_proj.

Device-side formulation (all matmuls bf16, fp32 accumulate):
  - host passes x[b].T so QKV projections contract C on partitions:
      qT/kT  = W.T @ x.T        -> [head_dim(=partitions), T]
      V'     = x @ [Wv|0] + ones-col -> [T(=partitions), 3*65]  (col 64 of
               each 65-block is constant 1 -> PV also yields softmax denoms)
  - scores computed transposed, S^T[k, q] via lhsT=kT, rhs=qT; two heads per
    512-cycle slot via PE row-tiling (K=64 each, concurrent row groups).
  - softmax without max-subtraction (scores are O(5), exp safe in fp32):
      P^T = exp(0.125 * S^T) on ScalarE, PSUM->SBUF bf16, one activate per
      [128, 1024] (both heads / both k-chunks of a slot share it).
  - causal: strictly-masked k-chunks never computed; at boundary tiles the
    score matmul / exp / PV shrink to the valid column subrange and only the
    diagonal [128,128] strip is multiplied by a triangular 0/1 bf16 mask.
  - PV: oT'[65, q] += V'[k,65].T @ P^T[k,q] accumulated over k-chunks; row 64
    is the softmax denominator (V' carries a constant-1 column; the V-bias
    commutes out of softmax and is added on the host as bv @ w_proj).
    Normalize: approx-reciprocal on DVE, GpSimd partition_broadcast across
    the 64 head-dim partitions, one DVE multiply -> yT bf16.
  - output projection (packed K=128): out[t,:] = yTa[:,t].T @ wp[h01-rows] +
    yT2[:,t].T @ wp[h2-rows];  h1's normalized slab is DMA-shifted into
    partitions 64..127 of yTa so two heads contract in one matmul.
  - QKV-projection and output-projection work is interleaved into the
    attention loop as PE filler ops so TensorE never idles (keeps the HAM
    clock gate at 2.4 GHz) while ScalarE streams the exps.
"""

import numpy as np

N_CORES = 8
B = 2
T = 4096
C = 768
NH = 12
HD = 64
HPC = 3            # heads per core
TCH = 512          # t / q chunk
KCH = 128          # k chunk
CPART = 128

_cache = {}


def _ensure_axon_hooks_module():
    """Make `from antenv.axon_hooks import ...` importable even on images
    whose antenv package lacks the module (profiling then degrades to a
    no-op instead of crashing run_bass_kernel_spmd(trace=True))."""
    import sys
    import types
    try:
        import antenv.axon_hooks  # noqa: F401
        return
    except Exception:
        pass
    m = types.ModuleType("antenv.axon_hooks")
    m._hook = None

    def _set(h):
        m._hook = h

    def _get():
        return m._hook

    m.set_axon_ntff_profile_hook = _set
    m.get_axon_ntff_profile_hook = _get
    sys.modules["antenv.axon_hooks"] = m


def build_program(t=T):
    """Build the single-core SPMD bass program (same program on all cores,
    per-core data). Returns the un-finalized Bacc."""
    import concourse.mybir as mybir
    import concourse.tile as tile
    from concourse import bacc
    from concourse.bass import ds, ts

    f32 = mybir.dt.float32
    bf16 = mybir.dt.bfloat16
    AF = mybir.ActivationFunctionType

    nt = t // TCH          # number of t/q chunks
    spk = TCH // KCH       # k-chunks per t-chunk (4)
    cc_n = C // CPART      # 6 contraction chunks

    nc = bacc.Bacc("TRN2", target_bir_lowering=False)

    # packed bf16 constants: [wq01 768 | wk01 768 | wqk2 768 | wv 1170 |
    #  wpA 768 | wpB 768 (rows 0:64) | tri 128 | misc 384 (row0: bv1+ones128)]
    PK_W = 6 * 128 * 3 + 6 * 195 + C + C + 128 + 384
    xT = nc.dram_tensor("xT", [128, (t // TCH) * (C // CPART) * TCH], bf16,
                        kind="ExternalInput")
    wpk_d = nc.dram_tensor("wpk", [128, PK_W], bf16, kind="ExternalInput")
    bpk_d = nc.dram_tensor("bpk", [128, 3], f32, kind="ExternalInput")
    out_d = nc.dram_tensor("out", [t, C], bf16, kind="ExternalOutput")

    with tile.TileContext(nc) as tc_:
        with (
            tc_.tile_pool(name="consts", bufs=1) as consts,
            tc_.tile_pool(name="big", bufs=1) as big,
            tc_.tile_pool(name="xin", bufs=3) as xin,
            tc_.tile_pool(name="ptp", bufs=8) as ptp,
            tc_.tile_pool(name="wkp", bufs=4) as wkp,
            tc_.tile_pool(name="sps", bufs=2, space="PSUM") as sps,
            tc_.tile_pool(name="ops", bufs=4, space="PSUM") as ops,
        ):
            # ---- init: packed weight DMAs (deferred until after the
            # first x-chunk DMA is issued) + one f32 bias DMA ----
            wpk = consts.tile([128, PK_W], bf16)
            bpk = consts.tile([128, 3], f32)

            def emit_weight_loads():
                nc.sync.dma_start(wpk[:, 0:768], wpk_d[:, 0:768])
                nc.sync.dma_start(bpk[:], bpk_d[:, :])
                nc.sync.dma_start(wpk[:, 768:2304], wpk_d[:, 768:2304])
                nc.sync.dma_start(wpk[:, 2304:3474], wpk_d[:, 2304:3474])
                nc.sync.dma_start(wpk[:, 3474:PK_W], wpk_d[:, 3474:PK_W])

            def seg(off, w):
                ap = wpk[:, off:off + w]
                return ap, off + w

            _o = 0
            wq01_f, _o = seg(_o, 6 * 128)
            wk01_f, _o = seg(_o, 6 * 128)
            wqk2_f, _o = seg(_o, 6 * 128)
            wv_f, _o = seg(_o, 6 * 195)
            wpA_sb, _o = seg(_o, C)
            wpB_full, _o = seg(_o, C)
            tri_sb, _o = seg(_o, 128)
            misc_f, _o = seg(_o, 384)
            wq01_sb = wq01_f.rearrange("p (c m) -> p c m", c=cc_n)
            wk01_sb = wk01_f.rearrange("p (c m) -> p c m", c=cc_n)
            wqk2_sb = wqk2_f.rearrange("p (c m) -> p c m", c=cc_n)
            wv_sb = wv_f.rearrange("p (c m) -> p c m", c=cc_n)
            wpB_sb = wpB_full[0:64, :]
            del misc_f  # reserved pack space, currently unused
            bq01_sb = bpk[:, 0:1]
            bk01_sb = bpk[:, 1:2]
            bqk2_sb = bpk[:, 2:3]

            # ---- persistent activations ----
            Q01 = big.tile([128, t], bf16)   # rows 0-63 qT_h0, 64-127 qT_h1
            K01 = big.tile([128, t], bf16)
            Q2 = big.tile([128, t], bf16)    # qT_h2 duplicated on both halves
            K2 = big.tile([128, t], bf16)
            Vp = big.tile([128, t // KCH, 195], bf16)
            yTa = big.tile([128, t], bf16)   # normalized h0 (0:64) | h1
            yT2 = big.tile([64, t], bf16)

            xT_r = xT[:, :].rearrange("p (nt c m) -> p nt c m", nt=nt,
                                      c=cc_n)

            # ---- QKV projection ops for one t-chunk (list of closures) ----
            def qkv_ops(tci):
                state = {}
                ops_l = []

                def dma_cast():
                    xtb = xin.tile([128, cc_n, TCH], bf16, tag="xtb",
                                   name="xtb")
                    if tci == 0:
                        for cc in range(cc_n):
                            nc.sync.dma_start(xtb[:, cc, :],
                                              xT_r[:, tci, cc, :])
                    else:
                        nc.sync.dma_start(xtb[:], xT_r[:, tci, :, :])
                    state["xtb"] = xtb
                ops_l.append(dma_cast)

                def qk_set(wsb, bsb, dst):
                    xtb = state["xtb"]
                    qkps = sps.tile([128, TCH], f32, tag="S", name="qkps")
                    for cc in range(cc_n):
                        nc.tensor.matmul(
                            qkps[:], wsb[:, cc, :], xtb[:, cc, :],
                            start=(cc == 0), stop=(cc == cc_n - 1))
                    if dst is None:
                        # packed [qT_h2; kT_h2]: bias-add the aligned halves
                        # into Q2/K2, then DMA-duplicate across halves.
                        nc.vector.tensor_scalar_add(
                            Q2[0:64, ts(tci, TCH)], qkps[0:64, :],
                            bsb[0:64, :])
                        nc.vector.tensor_scalar_add(
                            K2[64:128, ts(tci, TCH)], qkps[64:128, :],
                            bsb[64:128, :])
                        nc.sync.dma_start(Q2[64:128, ts(tci, TCH)],
                                          Q2[0:64, ts(tci, TCH)])
                        nc.sync.dma_start(K2[0:64, ts(tci, TCH)],
                                          K2[64:128, ts(tci, TCH)])
                    else:
                        nc.vector.tensor_scalar_add(
                            dst[:, ts(tci, TCH)], qkps[:], bsb[:])

                for wsb, bsb, dst in (
                    (wq01_sb, bq01_sb, Q01),
                    (wk01_sb, bk01_sb, K01),
                ):
                    ops_l.append(
                        lambda w=wsb, b=bsb, d=dst: qk_set(w, b, d))

                def v_set(st):
                    xtb = state["xtb"]
                    tt = tci * spk + st
                    vps = ops.tile([128, 195], f32, tag="oT", name="vps")
                    for cc in range(cc_n):
                        nc.tensor.matmul(
                            vps[:], xtb[:, cc, ts(st, 128)], wv_sb[:, cc, :],
                            start=(cc == 0), stop=(cc == cc_n - 1))
                    nc.vector.tensor_copy(Vp[:, tt, :], vps[:])
                    # denominator ones columns (cols 64/129/194 of each row)
                    nc.vector.memset(
                        Vp[:, tt, :].rearrange("p (a b) -> p a b", b=65)[
                            :, :, 64], 1.0)

                for st in range(spk):
                    ops_l.append(lambda s=st: v_set(s))
                # Q2/K2 are only consumed by pass 2 -- emit last so the next
                # chunk's pass 1 (which needs Q01/K01/V) can start sooner.
                ops_l.append(lambda: qk_set(wqk2_sb, bqk2_sb, None))
                return ops_l

            # ---- output-projection ops for one t-chunk ----
            def proj_ops(tci):
                def do_tile(tt):
                    po1 = ops.tile([128, 512], f32, tag="oT", name="po1")
                    po2 = ops.tile([128, 256], f32, tag="oT", name="po2")
                    for po, cs, cw in ((po1, 0, 512), (po2, 512, 256)):
                        nc.tensor.matmul(po[:], yTa[:, ts(tt, 128)],
                                         wpA_sb[:, ds(cs, cw)],
                                         start=True, stop=False)
                        nc.tensor.matmul(po[:], yT2[:, ts(tt, 128)],
                                         wpB_sb[:, ds(cs, cw)],
                                         start=False, stop=True)
                    pout = xin.tile([128, C], bf16, tag="pout", name="pout")
                    nc.vector.tensor_copy(pout[:, 0:512], po1[:])
                    nc.vector.tensor_copy(pout[:, 512:768], po2[:])
                    nc.sync.dma_start(out_d[ts(tt, 128), :], pout[:])

                return [lambda x=(tci * spk + s): do_tile(x)
                        for s in range(spk)]

            # ---- attention ----
            def normalize(oT, h, qc):
                den = wkp.tile([1, TCH], f32, tag="den", name="den")
                nc.vector.tensor_copy(den[:], oT[64:65, :])
                recip = wkp.tile([1, TCH], f32, tag="recip", name="recip")
                nc.vector.reciprocal_approx_fast(out=recip[:], in_=den[:])
                rb = wkp.tile([64, TCH], f32, tag="rb", name="rb")
                nc.gpsimd.partition_broadcast(rb[:], recip[:])
                if h == 0:
                    nc.vector.tensor_mul(yTa[0:64, ts(qc, TCH)], oT[0:64, :],
                                         rb[:])
                elif h == 2:
                    nc.vector.tensor_mul(yT2[0:64, ts(qc, TCH)], oT[0:64, :],
                                         rb[:])
                else:
                    y1t = wkp.tile([64, TCH], bf16, tag="y1t", name="y1t")
                    nc.vector.tensor_mul(y1t[:], oT[0:64, :], rb[:])
                    nc.sync.dma_start(yTa[64:128, ts(qc, TCH)], y1t[:])

            def attention(qc, fillers):
                nkc = (qc + 1) * spk
                q0 = qc * TCH

                def lo_of(kc):
                    m = kc - qc * spk
                    return max(0, 128 * m), m

                # ---- pass 1: heads 0,1 row-tiled ----
                oT0 = ops.tile([65, TCH], f32, tag="oT", name="oT0")
                oT1 = ops.tile([65, TCH], f32, tag="oT", name="oT1")
                s_pend = {}

                def emit_s01(kc):
                    lo, _ = lo_of(kc)
                    S = sps.tile([128, 1024], f32, tag="S", name="S01")
                    nc.tensor.matmul(
                        S[:, lo:TCH],
                        K01[0:64, ts(kc, KCH)], Q01[0:64, ds(q0 + lo,
                                                             TCH - lo)],
                        start=True, stop=True, tile_position=(0, 0))
                    nc.tensor.matmul(
                        S[:, TCH + lo:1024],
                        K01[64:128, ts(kc, KCH)], Q01[64:128, ds(q0 + lo,
                                                                 TCH - lo)],
                        start=True, stop=True, tile_position=(64, 0))
                    s_pend[kc] = S

                emit_s01(0)
                if nkc > 1:
                    emit_s01(1)
                for kc in range(nkc):
                    lo, m = lo_of(kc)
                    S = s_pend.pop(kc)
                    PT = ptp.tile([128, 1024], bf16, tag="PT", name="PT")
                    if lo == 0:
                        nc.scalar.activation(PT[:], S[:], AF.Exp, scale=0.125)
                    else:
                        # both halves have the same valid width: one strided
                        # activate covers [lo:TCH] and [TCH+lo:1024]
                        s_v = S[:].rearrange("p (h q) -> p h q", h=2)[
                            :, :, lo:TCH]
                        p_v = PT[:].rearrange("p (h q) -> p h q", h=2)[
                            :, :, lo:TCH]
                        nc.scalar.activation(p_v, s_v, AF.Exp, scale=0.125)
                    if kc + 2 < nkc:
                        emit_s01(kc + 2)
                    if m >= 0:
                        nc.vector.tensor_mul(PT[:, ds(lo, 128)],
                                             PT[:, ds(lo, 128)], tri_sb[:])
                        nc.vector.tensor_mul(PT[:, ds(TCH + lo, 128)],
                                             PT[:, ds(TCH + lo, 128)],
                                             tri_sb[:])
                    nc.tensor.matmul(oT0[:, lo:TCH], Vp[:, kc, 0:65],
                                     PT[:, lo:TCH],
                                     start=(kc == 0), stop=(kc == nkc - 1))
                    nc.tensor.matmul(oT1[:, lo:TCH], Vp[:, kc, 65:130],
                                     PT[:, TCH + lo:1024],
                                     start=(kc == 0), stop=(kc == nkc - 1))
                    if fillers:
                        fillers.pop(0)()
                normalize(oT0, 0, qc)
                normalize(oT1, 1, qc)

                # ---- pass 2: head 2, k-chunk pairs row-tiled ----
                oT2 = ops.tile([65, TCH], f32, tag="oT", name="oT2")
                npair = nkc // 2
                s2_pend = {}

                def emit_s2(kp):
                    kc0, kc1 = 2 * kp, 2 * kp + 1
                    lo0, _ = lo_of(kc0)
                    lo1, _ = lo_of(kc1)
                    S2 = sps.tile([128, 1024], f32, tag="S", name="S2")
                    nc.tensor.matmul(
                        S2[:, lo0:TCH],
                        K2[0:64, ts(kc0, KCH)], Q2[0:64, ds(q0 + lo0,
                                                            TCH - lo0)],
                        start=True, stop=True, tile_position=(0, 0))
                    nc.tensor.matmul(
                        S2[:, TCH + lo1:1024],
                        K2[64:128, ts(kc1, KCH)], Q2[64:128, ds(q0 + lo1,
                                                                TCH - lo1)],
                        start=True, stop=True, tile_position=(64, 0))
                    s2_pend[kp] = S2

                emit_s2(0)
                if npair > 1:
                    emit_s2(1)
                for kp in range(npair):
                    kc0, kc1 = 2 * kp, 2 * kp + 1
                    lo0, m0 = lo_of(kc0)
                    lo1, m1 = lo_of(kc1)
                    S2 = s2_pend.pop(kp)
                    PT2 = ptp.tile([128, 1024], bf16, tag="PT", name="PT2")
                    if lo0 == 0 and lo1 == 0:
                        nc.scalar.activation(PT2[:], S2[:], AF.Exp,
                                             scale=0.125)
                    else:
                        nc.scalar.activation(PT2[:, lo0:TCH], S2[:, lo0:TCH],
                                             AF.Exp, scale=0.125)
                        nc.scalar.activation(PT2[:, TCH + lo1:1024],
                                             S2[:, TCH + lo1:1024],
                                             AF.Exp, scale=0.125)
                    if kp + 2 < npair:
                        emit_s2(kp + 2)
                    if m0 >= 0:
                        nc.vector.tensor_mul(PT2[:, ds(lo0, 128)],
                                             PT2[:, ds(lo0, 128)], tri_sb[:])
                    if m1 >= 0:
                        nc.vector.tensor_mul(PT2[:, ds(TCH + lo1, 128)],
                                             PT2[:, ds(TCH + lo1, 128)],
                                             tri_sb[:])
                    nc.tensor.matmul(oT2[:, lo0:TCH], Vp[:, kc0, 130:195],
                                     PT2[:, lo0:TCH],
                                     start=(kp == 0), stop=False)
                    nc.tensor.matmul(oT2[:, lo1:TCH], Vp[:, kc1, 130:195],
                                     PT2[:, TCH + lo1:1024],
                                     start=False, stop=(kp == npair - 1))
                    if fillers:
                        fillers.pop(0)()
                normalize(oT2, 2, qc)

            # ---- main schedule: QKV(0) up front, then per-qc attention
            # with next-chunk QKV + prev-chunk proj injected as PE fillers
            ops0 = qkv_ops(0)
            ops0[0]()
            emit_weight_loads()
            for op in ops0[1:]:
                op()
            for qc in range(nt):
                fillers = []
                if qc + 1 < nt:
                    fillers += qkv_ops(qc + 1)
                if qc >= 1:
                    fillers += proj_ops(qc - 1)
                attention(qc, fillers)
                for op in fillers:
                    op()
            for op in proj_ops(nt - 1):
                op()

    return nc


def arrange_x(xb):
    """x[b] is [t, C]; device wants xT as [128, nt, cc, TCH] contiguous."""
    import ml_dtypes
    t = xb.shape[0]
    xt = xb.T.reshape(C // CPART, CPART, t // TCH, TCH)
    xt = xt.transpose(1, 2, 0, 3).reshape(CPART, -1)
    return np.ascontiguousarray(xt).astype(ml_dtypes.bfloat16)


def make_tri():
    import ml_dtypes
    p = np.arange(128)[:, None]
    j = np.arange(128)[None, :]
    return (j - p >= 0).astype(ml_dtypes.bfloat16)


def core_inputs(c, x, w_attn, b_attn, w_proj, xT_by_batch, tri):
    import ml_dtypes
    f32 = np.float32
    b = c // 4
    heads = [(c % 4) * HPC + i for i in range(HPC)]
    h0, h1, h2 = heads

    def Wq(h):
        return w_attn[:, h * HD:(h + 1) * HD]

    def Wk(h):
        return w_attn[:, C + h * HD:C + (h + 1) * HD]

    def Wv(h):
        return w_attn[:, 2 * C + h * HD:2 * C + (h + 1) * HD]

    def bq(h):
        return b_attn[h * HD:(h + 1) * HD]

    def bk(h):
        return b_attn[C + h * HD:C + (h + 1) * HD]

    def bv(h):
        return b_attn[2 * C + h * HD:2 * C + (h + 1) * HD]

    wv195 = np.zeros((C, 195), f32)
    for i, h in enumerate(heads):
        wv195[:, i * 65:i * 65 + 64] = Wv(h)
    bf = ml_dtypes.bfloat16

    def arr(w):
        m = w.shape[1]
        return np.ascontiguousarray(
            w.reshape(C // CPART, CPART, m).transpose(1, 0, 2).reshape(
                CPART, -1)).astype(bf)

    wp192 = np.concatenate([w_proj[h * HD:(h + 1) * HD, :] for h in heads], 0)
    wpB = np.zeros((CPART, C), np.float32)
    wpB[0:64, :] = wp192[128:192, :]
    misc = np.zeros((CPART, 384), np.float32)
    wpk = np.concatenate([
        arr(np.concatenate([Wq(h0), Wq(h1)], 1)).astype(np.float32),
        arr(np.concatenate([Wk(h0), Wk(h1)], 1)).astype(np.float32),
        arr(np.concatenate([Wq(h2), Wk(h2)], 1)).astype(np.float32),
        arr(wv195).astype(np.float32),
        wp192[0:128, :], wpB, tri.astype(np.float32), misc,
    ], axis=1).astype(bf)
    bpk = np.stack([
        np.concatenate([bq(h0), bq(h1)]),
        np.concatenate([bk(h0), bk(h1)]),
        np.concatenate([bq(h2), bk(h2)]),
    ], axis=1).astype(np.float32)
    return {
        "xT": xT_by_batch[b],
        "wpk": np.ascontiguousarray(wpk),
        "bpk": np.ascontiguousarray(bpk),
    }


TRACE = False
LAST_EXEC_NS = None
LAST_RESULTS = None


def kernel(x, w_attn, b_attn, w_proj, b_proj):
    global LAST_EXEC_NS, LAST_RESULTS
    _ensure_axon_hooks_module()
    from concourse.bass_utils import run_bass_kernel_spmd

    x = np.asarray(x, np.float32)
    w_attn = np.asarray(w_attn, np.float32)
    b_attn = np.asarray(b_attn, np.float32)
    w_proj = np.asarray(w_proj, np.float32)
    b_proj = np.asarray(b_proj, np.float32)

    if "nc" not in _cache:
        nc = build_program()
        nc.finalize()
        _cache["nc"] = nc
    nc = _cache["nc"]

    import ml_dtypes  # noqa: F401
    xT_by_batch = [arrange_x(x[b]) for b in range(B)]
    tri = make_tri()
    in_maps = [
        core_inputs(c, x, w_attn, b_attn, w_proj, xT_by_batch, tri)
        for c in range(N_CORES)
    ]
    res = run_bass_kernel_spmd(nc, in_maps, core_ids=list(range(N_CORES)),
                               trace=TRACE)
    LAST_EXEC_NS = res.exec_time_ns
    LAST_RESULTS = res
    out = np.zeros((B, T, C), np.float32)
    for c in range(N_CORES):
        out[c // 4] += np.asarray(res.results[c]["out"], np.float32)
    # V-bias commutes out of softmax (weights sum to 1): add bv @ w_proj
    bv_all = b_attn[2 * C:]
    out += (b_proj + bv_all @ w_proj)[None, None, :]
    return out


# revision 29
# speedup vs baseline: 1.0242x; 1.0242x over previous
"""Causal self-attention (GPT-style, B=2, T=4096, C=768, 12 heads) on 8 TRN2
NeuronCores.

Sharding: core c handles batch b = c//4 and heads [3g, 3g+1, 3g+2] with
g = c%4 (data parallel on B x tensor parallel on heads).  Each core computes
its heads' attention output projected through its slice of w_proj; the host
sums the 4 partial [T, C] outputs per batch and adds b_proj.

Device-side formulation (all matmuls bf16, fp32 accumulate):
  - host passes x[b].T so QKV projections contract C on partitions:
      qT/kT  = W.T @ x.T        -> [head_dim(=partitions), T]
      V'     = x @ [Wv|0] + ones-col -> [T(=partitions), 3*65]  (col 64 of
               each 65-block is constant 1 -> PV also yields softmax denoms)
  - scores computed transposed, S^T[k, q] via lhsT=kT, rhs=qT; two heads per
    512-cycle slot via PE row-tiling (K=64 each, concurrent row groups).
  - softmax without max-subtraction (scores are O(5), exp safe in fp32):
      P^T = exp(0.125 * S^T) on ScalarE, PSUM->SBUF bf16, one activate per
      [128, 1024] (both heads / both k-chunks of a slot share it).
  - causal: strictly-masked k-chunks never computed; at boundary tiles the
    score matmul / exp / PV shrink to the valid column subrange and only the
    diagonal [128,128] strip is multiplied by a triangular 0/1 bf16 mask.
  - PV: oT'[65, q] += V'[k,65].T @ P^T[k,q] accumulated over k-chunks; row 64
    is the softmax denominator (V' carries a constant-1 column; the V-bias
    commutes out of softmax and is added on the host as bv @ w_proj).
    Normalize: approx-reciprocal on DVE, GpSimd partition_broadcast across
    the 64 head-dim partitions, one DVE multiply -> yT bf16.
  - output projection (packed K=128): out[t,:] = yTa[:,t].T @ wp[h01-rows] +
    yT2[:,t].T @ wp[h2-rows];  h1's normalized slab is DMA-shifted into
    partitions 64..127 of yTa so two heads contract in one matmul.
  - QKV-projection and output-projection work is interleaved into the
    attention loop as PE filler ops so TensorE never idles (keeps the HAM
    clock gate at 2.4 GHz) while ScalarE streams the exps.
"""

import numpy as np

N_CORES = 8
B = 2
T = 4096
C = 768
NH = 12
HD = 64
HPC = 3            # heads per core
TCH = 512          # t / q chunk
KCH = 128          # k chunk
CPART = 128

_cache = {}


def _ensure_axon_hooks_module():
    """Make `from antenv.axon_hooks import ...` importable even on images
    whose antenv package lacks the module (profiling then degrades to a
    no-op instead of crashing run_bass_kernel_spmd(trace=True))."""
    import sys
    import types
    try:
        import antenv.axon_hooks  # noqa: F401
        return
    except Exception:
        pass
    m = types.ModuleType("antenv.axon_hooks")
    m._hook = None

    def _set(h):
        m._hook = h

    def _get():
        return m._hook

    m.set_axon_ntff_profile_hook = _set
    m.get_axon_ntff_profile_hook = _get
    sys.modules["antenv.axon_hooks"] = m


def build_program(t=T):
    """Build the single-core SPMD bass program (same program on all cores,
    per-core data). Returns the un-finalized Bacc."""
    import concourse.mybir as mybir
    import concourse.tile as tile
    from concourse import bacc
    from concourse.bass import ds, ts

    f32 = mybir.dt.float32
    bf16 = mybir.dt.bfloat16
    AF = mybir.ActivationFunctionType

    nt = t // TCH          # number of t/q chunks
    spk = TCH // KCH       # k-chunks per t-chunk (4)
    cc_n = C // CPART      # 6 contraction chunks

    nc = bacc.Bacc("TRN2", target_bir_lowering=False)

    # packed bf16 constants: [wq01 768 | wk01 768 | wqk2 768 | wv 1170 |
    #  wpA 768 | wpB 768 (rows 0:64) | tri 128 | misc 384 (row0: bv1+ones128)]
    PK_W = 6 * 128 * 3 + 6 * 195 + C + C + 128 + 384
    xT = nc.dram_tensor("xT", [128, (t // TCH) * (C // CPART) * TCH], bf16,
                        kind="ExternalInput")
    wpk_d = nc.dram_tensor("wpk", [128, PK_W], bf16, kind="ExternalInput")
    bpk_d = nc.dram_tensor("bpk", [128, 3], f32, kind="ExternalInput")
    out_d = nc.dram_tensor("out", [t, C], bf16, kind="ExternalOutput")

    with tile.TileContext(nc) as tc_:
        with (
            tc_.tile_pool(name="consts", bufs=1) as consts,
            tc_.tile_pool(name="big", bufs=1) as big,
            tc_.tile_pool(name="xin", bufs=3) as xin,
            tc_.tile_pool(name="ptp", bufs=8) as ptp,
            tc_.tile_pool(name="wkp", bufs=4) as wkp,
            tc_.tile_pool(name="sps", bufs=2, space="PSUM") as sps,
            tc_.tile_pool(name="ops", bufs=4, space="PSUM") as ops,
        ):
            # ---- init: packed weight DMAs (deferred until after the
            # first x-chunk DMA is issued) + one f32 bias DMA ----
            wpk = consts.tile([128, PK_W], bf16)
            bpk = consts.tile([128, 3], f32)

            def emit_weight_loads():
                nc.sync.dma_start(wpk[:, 0:768], wpk_d[:, 0:768])
                nc.sync.dma_start(bpk[:], bpk_d[:, :])
                nc.sync.dma_start(wpk[:, 768:2304], wpk_d[:, 768:2304])
                nc.sync.dma_start(wpk[:, 2304:3474], wpk_d[:, 2304:3474])
                nc.sync.dma_start(wpk[:, 3474:PK_W], wpk_d[:, 3474:PK_W])

            def seg(off, w):
                ap = wpk[:, off:off + w]
                return ap, off + w

            _o = 0
            wq01_f, _o = seg(_o, 6 * 128)
            wk01_f, _o = seg(_o, 6 * 128)
            wqk2_f, _o = seg(_o, 6 * 128)
            wv_f, _o = seg(_o, 6 * 195)
            wpA_sb, _o = seg(_o, C)
            wpB_full, _o = seg(_o, C)
            tri_sb, _o = seg(_o, 128)
            misc_f, _o = seg(_o, 384)
            wq01_sb = wq01_f.rearrange("p (c m) -> p c m", c=cc_n)
            wk01_sb = wk01_f.rearrange("p (c m) -> p c m", c=cc_n)
            wqk2_sb = wqk2_f.rearrange("p (c m) -> p c m", c=cc_n)
            wv_sb = wv_f.rearrange("p (c m) -> p c m", c=cc_n)
            wpB_sb = wpB_full[0:64, :]
            del misc_f  # reserved pack space, currently unused
            bq01_sb = bpk[:, 0:1]
            bk01_sb = bpk[:, 1:2]
            bqk2_sb = bpk[:, 2:3]

            # ---- persistent activations ----
            Q01 = big.tile([128, t], bf16)   # rows 0-63 qT_h0, 64-127 qT_h1
            K01 = big.tile([128, t], bf16)
            Q2 = big.tile([128, t], bf16)    # qT_h2 duplicated on both halves
            K2 = big.tile([128, t], bf16)
            Vp = big.tile([128, t // KCH, 195], bf16)
            yTa = big.tile([128, t], bf16)   # normalized h0 (0:64) | h1
            yT2 = big.tile([64, t], bf16)

            xT_r = xT[:, :].rearrange("p (nt c m) -> p nt c m", nt=nt,
                                      c=cc_n)

            # ---- QKV projection ops for one t-chunk (list of closures) ----
            def qkv_ops(tci):
                state = {}
                ops_l = []

                def dma_cast():
                    xtb = xin.tile([128, cc_n, TCH], bf16, tag="xtb",
                                   name="xtb")
                    if tci == 0:
                        for cc in range(cc_n):
                            nc.sync.dma_start(xtb[:, cc, :],
                                              xT_r[:, tci, cc, :])
                    else:
                        nc.sync.dma_start(xtb[:], xT_r[:, tci, :, :])
                    state["xtb"] = xtb
                ops_l.append(dma_cast)

                def qk_set(wsb, bsb, dst):
                    xtb = state["xtb"]
                    qkps = sps.tile([128, TCH], f32, tag="S", name="qkps")
                    for cc in range(cc_n):
                        nc.tensor.matmul(
                            qkps[:], wsb[:, cc, :], xtb[:, cc, :],
                            start=(cc == 0), stop=(cc == cc_n - 1))
                    if dst is None:
                        # packed [qT_h2; kT_h2]: bias-add the aligned halves
                        # into Q2/K2, then DMA-duplicate across halves.
                        nc.vector.tensor_scalar_add(
                            Q2[0:64, ts(tci, TCH)], qkps[0:64, :],
                            bsb[0:64, :])
                        nc.vector.tensor_scalar_add(
                            K2[64:128, ts(tci, TCH)], qkps[64:128, :],
                            bsb[64:128, :])
                        nc.sync.dma_start(Q2[64:128, ts(tci, TCH)],
                                          Q2[0:64, ts(tci, TCH)])
                        nc.sync.dma_start(K2[0:64, ts(tci, TCH)],
                                          K2[64:128, ts(tci, TCH)])
                    else:
                        nc.vector.tensor_scalar_add(
                            dst[:, ts(tci, TCH)], qkps[:], bsb[:])

                for wsb, bsb, dst in (
                    (wq01_sb, bq01_sb, Q01),
                    (wk01_sb, bk01_sb, K01),
                    (wqk2_sb, bqk2_sb, None),
                ):
                    ops_l.append(
                        lambda w=wsb, b=bsb, d=dst: qk_set(w, b, d))

                def v_set(st):
                    xtb = state["xtb"]
                    tt = tci * spk + st
                    vps = ops.tile([128, 195], f32, tag="oT", name="vps")
                    for cc in range(cc_n):
                        nc.tensor.matmul(
                            vps[:], xtb[:, cc, ts(st, 128)], wv_sb[:, cc, :],
                            start=(cc == 0), stop=(cc == cc_n - 1))
                    nc.vector.tensor_copy(Vp[:, tt, :], vps[:])
                    # denominator ones columns (cols 64/129/194 of each row)
                    nc.vector.memset(
                        Vp[:, tt, :].rearrange("p (a b) -> p a b", b=65)[
                            :, :, 64], 1.0)

                for st in range(spk):
                    ops_l.append(lambda s=st: v_set(s))
                return ops_l

            # ---- output-projection ops for one t-chunk ----
            def proj_ops(tci):
                def do_tile(tt):
                    po1 = ops.tile([128, 512], f32, tag="oT", name="po1")
                    po2 = ops.tile([128, 256], f32, tag="oT", name="po2")
                    for po, cs, cw in ((po1, 0, 512), (po2, 512, 256)):
                        nc.tensor.matmul(po[:], yTa[:, ts(tt, 128)],
                                         wpA_sb[:, ds(cs, cw)],
                                         start=True, stop=False)
                        nc.tensor.matmul(po[:], yT2[:, ts(tt, 128)],
                                         wpB_sb[:, ds(cs, cw)],
                                         start=False, stop=True)
                    pout = xin.tile([128, C], bf16, tag="pout", name="pout")
                    nc.vector.tensor_copy(pout[:, 0:512], po1[:])
                    nc.vector.tensor_copy(pout[:, 512:768], po2[:])
                    nc.sync.dma_start(out_d[ts(tt, 128), :], pout[:])

                return [lambda x=(tci * spk + s): do_tile(x)
                        for s in range(spk)]

            # ---- attention ----
            def normalize(oT, h, qc):
                den = wkp.tile([1, TCH], f32, tag="den", name="den")
                nc.vector.tensor_copy(den[:], oT[64:65, :])
                recip = wkp.tile([1, TCH], f32, tag="recip", name="recip")
                nc.vector.reciprocal_approx_fast(out=recip[:], in_=den[:])
                rb = wkp.tile([64, TCH], f32, tag="rb", name="rb")
                nc.gpsimd.partition_broadcast(rb[:], recip[:])
                if h == 0:
                    nc.vector.tensor_mul(yTa[0:64, ts(qc, TCH)], oT[0:64, :],
                                         rb[:])
                elif h == 2:
                    nc.vector.tensor_mul(yT2[0:64, ts(qc, TCH)], oT[0:64, :],
                                         rb[:])
                else:
                    y1t = wkp.tile([64, TCH], bf16, tag="y1t", name="y1t")
                    nc.vector.tensor_mul(y1t[:], oT[0:64, :], rb[:])
                    nc.sync.dma_start(yTa[64:128, ts(qc, TCH)], y1t[:])

            def attention(qc, fillers):
                nkc = (qc + 1) * spk
                q0 = qc * TCH

                def lo_of(kc):
                    m = kc - qc * spk
                    return max(0, 128 * m), m

                # ---- pass 1: heads 0,1 row-tiled ----
                oT0 = ops.tile([65, TCH], f32, tag="oT", name="oT0")
                oT1 = ops.tile([65, TCH], f32, tag="oT", name="oT1")
                s_pend = {}

                def emit_s01(kc):
                    lo, _ = lo_of(kc)
                    S = sps.tile([128, 1024], f32, tag="S", name="S01")
                    nc.tensor.matmul(
                        S[:, lo:TCH],
                        K01[0:64, ts(kc, KCH)], Q01[0:64, ds(q0 + lo,
                                                             TCH - lo)],
                        start=True, stop=True, tile_position=(0, 0))
                    nc.tensor.matmul(
                        S[:, TCH + lo:1024],
                        K01[64:128, ts(kc, KCH)], Q01[64:128, ds(q0 + lo,
                                                                 TCH - lo)],
                        start=True, stop=True, tile_position=(64, 0))
                    s_pend[kc] = S

                emit_s01(0)
                if nkc > 1:
                    emit_s01(1)
                for kc in range(nkc):
                    lo, m = lo_of(kc)
                    S = s_pend.pop(kc)
                    PT = ptp.tile([128, 1024], bf16, tag="PT", name="PT")
                    if lo == 0:
                        nc.scalar.activation(PT[:], S[:], AF.Exp, scale=0.125)
                    else:
                        # both halves have the same valid width: one strided
                        # activate covers [lo:TCH] and [TCH+lo:1024]
                        s_v = S[:].rearrange("p (h q) -> p h q", h=2)[
                            :, :, lo:TCH]
                        p_v = PT[:].rearrange("p (h q) -> p h q", h=2)[
                            :, :, lo:TCH]
                        nc.scalar.activation(p_v, s_v, AF.Exp, scale=0.125)
                    if kc + 2 < nkc:
                        emit_s01(kc + 2)
                    if m >= 0:
                        nc.vector.tensor_mul(PT[:, ds(lo, 128)],
                                             PT[:, ds(lo, 128)], tri_sb[:])
                        nc.vector.tensor_mul(PT[:, ds(TCH + lo, 128)],
                                             PT[:, ds(TCH + lo, 128)],
                                             tri_sb[:])
                    nc.tensor.matmul(oT0[:, lo:TCH], Vp[:, kc, 0:65],
                                     PT[:, lo:TCH],
                                     start=(kc == 0), stop=(kc == nkc - 1))
                    nc.tensor.matmul(oT1[:, lo:TCH], Vp[:, kc, 65:130],
                                     PT[:, TCH + lo:1024],
                                     start=(kc == 0), stop=(kc == nkc - 1))
                    if fillers:
                        fillers.pop(0)()
                normalize(oT0, 0, qc)
                normalize(oT1, 1, qc)

                # ---- pass 2: head 2, k-chunk pairs row-tiled ----
                oT2 = ops.tile([65, TCH], f32, tag="oT", name="oT2")
                npair = nkc // 2
                s2_pend = {}

                def emit_s2(kp):
                    kc0, kc1 = 2 * kp, 2 * kp + 1
                    lo0, _ = lo_of(kc0)
                    lo1, _ = lo_of(kc1)
                    S2 = sps.tile([128, 1024], f32, tag="S", name="S2")
                    nc.tensor.matmul(
                        S2[:, lo0:TCH],
                        K2[0:64, ts(kc0, KCH)], Q2[0:64, ds(q0 + lo0,
                                                            TCH - lo0)],
                        start=True, stop=True, tile_position=(0, 0))
                    nc.tensor.matmul(
                        S2[:, TCH + lo1:1024],
                        K2[64:128, ts(kc1, KCH)], Q2[64:128, ds(q0 + lo1,
                                                                TCH - lo1)],
                        start=True, stop=True, tile_position=(64, 0))
                    s2_pend[kp] = S2

                emit_s2(0)
                if npair > 1:
                    emit_s2(1)
                for kp in range(npair):
                    kc0, kc1 = 2 * kp, 2 * kp + 1
                    lo0, m0 = lo_of(kc0)
                    lo1, m1 = lo_of(kc1)
                    S2 = s2_pend.pop(kp)
                    PT2 = ptp.tile([128, 1024], bf16, tag="PT", name="PT2")
                    if lo0 == 0 and lo1 == 0:
                        nc.scalar.activation(PT2[:], S2[:], AF.Exp,
                                             scale=0.125)
                    else:
                        nc.scalar.activation(PT2[:, lo0:TCH], S2[:, lo0:TCH],
                                             AF.Exp, scale=0.125)
                        nc.scalar.activation(PT2[:, TCH + lo1:1024],
                                             S2[:, TCH + lo1:1024],
                                             AF.Exp, scale=0.125)
                    if kp + 2 < npair:
                        emit_s2(kp + 2)
                    if m0 >= 0:
                        nc.vector.tensor_mul(PT2[:, ds(lo0, 128)],
                                             PT2[:, ds(lo0, 128)], tri_sb[:])
                    if m1 >= 0:
                        nc.vector.tensor_mul(PT2[:, ds(TCH + lo1, 128)],
                                             PT2[:, ds(TCH + lo1, 128)],
                                             tri_sb[:])
                    nc.tensor.matmul(oT2[:, lo0:TCH], Vp[:, kc0, 130:195],
                                     PT2[:, lo0:TCH],
                                     start=(kp == 0), stop=False)
                    nc.tensor.matmul(oT2[:, lo1:TCH], Vp[:, kc1, 130:195],
                                     PT2[:, TCH + lo1:1024],
                                     start=False, stop=(kp == npair - 1))
                    if fillers:
                        fillers.pop(0)()
                normalize(oT2, 2, qc)

            # ---- main schedule: QKV(0) up front, then per-qc attention
            # with next-chunk QKV + prev-chunk proj injected as PE fillers
            ops0 = qkv_ops(0)
            ops0[0]()
            emit_weight_loads()
            for op in ops0[1:]:
                op()
            for qc in range(nt):
                fillers = []
                if qc + 1 < nt:
                    fillers += qkv_ops(qc + 1)
                if qc >= 1:
                    fillers += proj_ops(qc - 1)
                attention(qc, fillers)
                for op in fillers:
                    op()
            for op in proj_ops(nt - 1):
                op()

    return nc


def arrange_x(xb):
    """x[b] is [t, C]; device wants xT as [128, nt, cc, TCH] contiguous."""
    import ml_dtypes
    t = xb.shape[0]
    xt = xb.T.reshape(C // CPART, CPART, t // TCH, TCH)
    xt = xt.transpose(1, 2, 0, 3).reshape(CPART, -1)
    return np.ascontiguousarray(xt).astype(ml_dtypes.bfloat16)


def make_tri():
    import ml_dtypes
    p = np.arange(128)[:, None]
    j = np.arange(128)[None, :]
    return (j - p >= 0).astype(ml_dtypes.bfloat16)


def core_inputs(c, x, w_attn, b_attn, w_proj, xT_by_batch, tri):
    import ml_dtypes
    f32 = np.float32
    b = c // 4
    heads = [(c % 4) * HPC + i for i in range(HPC)]
    h0, h1, h2 = heads

    def Wq(h):
        return w_attn[:, h * HD:(h + 1) * HD]

    def Wk(h):
        return w_attn[:, C + h * HD:C + (h + 1) * HD]

    def Wv(h):
        return w_attn[:, 2 * C + h * HD:2 * C + (h + 1) * HD]

    def bq(h):
        return b_attn[h * HD:(h + 1) * HD]

    def bk(h):
        return b_attn[C + h * HD:C + (h + 1) * HD]

    def bv(h):
        return b_attn[2 * C + h * HD:2 * C + (h + 1) * HD]

    wv195 = np.zeros((C, 195), f32)
    for i, h in enumerate(heads):
        wv195[:, i * 65:i * 65 + 64] = Wv(h)
    bf = ml_dtypes.bfloat16

    def arr(w):
        m = w.shape[1]
        return np.ascontiguousarray(
            w.reshape(C // CPART, CPART, m).transpose(1, 0, 2).reshape(
                CPART, -1)).astype(bf)

    wp192 = np.concatenate([w_proj[h * HD:(h + 1) * HD, :] for h in heads], 0)
    wpB = np.zeros((CPART, C), np.float32)
    wpB[0:64, :] = wp192[128:192, :]
    misc = np.zeros((CPART, 384), np.float32)
    wpk = np.concatenate([
        arr(np.concatenate([Wq(h0), Wq(h1)], 1)).astype(np.float32),
        arr(np.concatenate([Wk(h0), Wk(h1)], 1)).astype(np.float32),
        arr(np.concatenate([Wq(h2), Wk(h2)], 1)).astype(np.float32),
        arr(wv195).astype(np.float32),
        wp192[0:128, :], wpB, tri.astype(np.float32), misc,
    ], axis=1).astype(bf)
    bpk = np.stack([
        np.concatenate([bq(h0), bq(h1)]),
        np.concatenate([bk(h0), bk(h1)]),
        np.concatenate([bq(h2), bk(h2)]),
    ], axis=1).astype(np.float32)
    return {
        "xT": xT_by_batch[b],
        "wpk": np.ascontiguousarray(wpk),
        "bpk": np.ascontiguousarray(bpk),
    }


TRACE = False
LAST_EXEC_NS = None
LAST_RESULTS = None


def kernel(x, w_attn, b_attn, w_proj, b_proj):
    global LAST_EXEC_NS, LAST_RESULTS
    _ensure_axon_hooks_module()
    from concourse.bass_utils import run_bass_kernel_spmd

    x = np.asarray(x, np.float32)
    w_attn = np.asarray(w_attn, np.float32)
    b_attn = np.asarray(b_attn, np.float32)
    w_proj = np.asarray(w_proj, np.float32)
    b_proj = np.asarray(b_proj, np.float32)

    if "nc" not in _cache:
        nc = build_program()
        nc.finalize()
        _cache["nc"] = nc
    nc = _cache["nc"]

    import ml_dtypes  # noqa: F401
    xT_by_batch = [arrange_x(x[b]) for b in range(B)]
    tri = make_tri()
    in_maps = [
        core_inputs(c, x, w_attn, b_attn, w_proj, xT_by_batch, tri)
        for c in range(N_CORES)
    ]
    res = run_bass_kernel_spmd(nc, in_maps, core_ids=list(range(N_CORES)),
                               trace=TRACE)
    LAST_EXEC_NS = res.exec_time_ns
    LAST_RESULTS = res
    out = np.zeros((B, T, C), np.float32)
    for c in range(N_CORES):
        out[c // 4] += np.asarray(res.results[c]["out"], np.float32)
    # V-bias commutes out of softmax (weights sum to 1): add bv @ w_proj
    bv_all = b_attn[2 * C:]
    out += (b_proj + bv_all @ w_proj)[None, None, :]
    return out
